# revision 1
# baseline (speedup 1.0000x reference)
"""Trainium kernel for nn_NewBackboneModel: conv stem -> primary point caps
(BN + global maxpool + squash) -> dynamic routing -> classifier head.

Strategy: the dominant compute (the [16,1024,128] x [B,128,N] primary-caps
einsum, ~34 GFLOP) is sharded over the 8 NeuronCores along the prim_caps
axis (P=1024 -> 128 per core) with jax shard_map; BN statistics for the
sharded branch are per-(q,p) and therefore fully local. The routing sum
over prim_caps ends with a single small psum (all-reduce) per routing
iteration, exactly as in the sharding hint. Everything is jit-compiled to
the NeuronCores via the XLA Neuron backend.
"""

import functools

import numpy as np
import jax
import jax.numpy as jnp
from jax.sharding import Mesh, PartitionSpec as P
from jax.experimental.shard_map import shard_map

BN_EPS = 1e-5
NUM_ROUTING_ITERS = 3
N_CORES = 8


def _bn_global(x, gamma, beta, axes):
    mean = x.mean(axes, keepdims=True)
    var = x.var(axes, keepdims=True)
    return (x - mean) * jax.lax.rsqrt(var + BN_EPS) * gamma + beta


def _squash(x):
    sn = jnp.sum(x * x, axis=-1, keepdims=True)
    return sn * x / ((1.0 + sn) * jnp.sqrt(sn))


def _sharded_body(x, w1, b1, g1, be1, w2, b2, g2, be2, w3, b3, g3, be3, Wr,
                  fc_w, fc_b):
    """Runs on every core. w3/b3/g3/be3/Wr arrive sharded on prim_caps axis
    (128 of 1024 rows per core); all other operands are replicated."""
    B = x.shape[0]

    # Conv stem (replicated on every core; cheap).
    h = jnp.einsum("oc,bcn->bon", w1, x) + b1[None, :, None]
    h = jax.nn.relu(_bn_global(h, g1[None, :, None], be1[None, :, None], (0, 2)))
    h = jnp.einsum("oc,bcn->bon", w2, h) + b2[None, :, None]
    h = jax.nn.relu(_bn_global(h, g2[None, :, None], be2[None, :, None], (0, 2)))

    # Primary caps: local p-shard of the big einsum. BN over (batch, points)
    # is per-(q,p) so it needs no cross-core communication.
    u = jnp.einsum("qpc,bcn->bqpn", w3, h) + b3[None, :, :, None]
    u = _bn_global(u, g3[None, :, :, None], be3[None, :, :, None], (0, 3))
    u = u.max(axis=-1)
    u = jnp.transpose(u, (0, 2, 1))  # [B, P/8, Q]
    u = _squash(u)

    # Routing. u_hat is [B, L, P/8, V] per core; softmax over L is local
    # (every core holds all L for its p's). Only the routing sum over
    # prim_caps crosses cores: one small [B, L, V] psum per iteration.
    u_hat = jnp.einsum("lpvq,bpq->blpv", Wr, u)
    L = Wr.shape[0]
    b_ij = jnp.zeros((B, L, u_hat.shape[2]), u_hat.dtype)
    v_j = None
    for _ in range(NUM_ROUTING_ITERS - 1):
        c_ij = jax.nn.softmax(b_ij, axis=1)
        s_j = jnp.einsum("blp,blpv->blv", c_ij, u_hat)
        s_j = jax.lax.psum(s_j, axis_name="core")
        v_j = _squash(s_j)
        b_ij = b_ij + jnp.einsum("blv,blpv->blp", v_j, u_hat)
    c_ij = jax.nn.softmax(b_ij, axis=1)
    s_j = jnp.einsum("blp,blpv->blv", c_ij, u_hat)
    s_j = jax.lax.psum(s_j, axis_name="core")
    v_j = _squash(s_j)  # [B, L, V], replicated

    presence = jnp.sqrt(jnp.sum(v_j * v_j, axis=2))
    logits = v_j.reshape(B, -1) @ fc_w.T + fc_b
    return logits, presence


@functools.cache
def _compiled():
    devices = jax.devices()[:N_CORES]
    mesh = Mesh(np.asarray(devices), ("core",))
    repl = P()
    shard_p = P("core")          # shard axis 0 (prim_caps)
    shard_p1 = P(None, "core")   # shard axis 1 (prim_caps)

    in_specs = (
        repl,                    # x
        repl, repl, repl, repl,  # w1 b1 g1 be1
        repl, repl, repl, repl,  # w2 b2 g2 be2
        shard_p1, shard_p1, shard_p1, shard_p1,  # w3 b3 g3 be3 ([Q, P, ...])
        shard_p1,                # Wr [L, P, V, Q]
        repl, repl,              # fc_w fc_b
    )
    out_specs = (repl, repl)

    fn = shard_map(
        _sharded_body,
        mesh=mesh,
        in_specs=in_specs,
        out_specs=out_specs,
        check_rep=False,
    )
    return jax.jit(fn)


def kernel(**inputs):
    order = ["x", "w1", "b1", "g1", "be1", "w2", "b2", "g2", "be2",
             "w3", "b3", "g3", "be3", "Wr", "fc_w", "fc_b"]
    args = [np.asarray(inputs[k], dtype=np.float32) for k in order]
    logits, presence = _compiled()(*args)
    return (np.asarray(logits), np.asarray(presence))


if __name__ == "__main__":
    rng = np.random.default_rng(0)
    # tiny self-smoke with proper shapes
    ins = {
        "x": rng.standard_normal((4, 3, 2048), dtype=np.float32),
        "w1": rng.standard_normal((64, 3), dtype=np.float32) * 0.02,
        "b1": np.zeros(64, np.float32),
        "g1": np.ones(64, np.float32),
        "be1": np.zeros(64, np.float32),
        "w2": rng.standard_normal((128, 64), dtype=np.float32) * 0.02,
        "b2": np.zeros(128, np.float32),
        "g2": np.ones(128, np.float32),
        "be2": np.zeros(128, np.float32),
        "w3": rng.standard_normal((16, 1024, 128), dtype=np.float32) * 0.02,
        "b3": np.zeros((16, 1024), np.float32),
        "g3": np.ones((16, 1024), np.float32),
        "be3": np.zeros((16, 1024), np.float32),
        "Wr": rng.standard_normal((64, 1024, 64, 16), dtype=np.float32) * 0.01,
        "fc_w": rng.standard_normal((40, 64 * 64), dtype=np.float32) * 0.02,
        "fc_b": np.zeros(40, np.float32),
    }
    out = kernel(**ins)
    print(out[0].shape, out[1].shape)
